# revision 31
# baseline (speedup 1.0000x reference)
"""Trainium2 Bass kernel for CausalSE (chunked-mean-pool -> per-channel EMA ->
int4-fake-quant SE bottleneck -> sigmoid gate -> gated residual).

Contract: kernel(**inputs) takes FULL unsharded inputs (as produced by
setup_inputs) and returns the FULL [16, 2048, 4096] float32 output.
Internally shards batch 16 -> 8 NeuronCores (2 per core), replicating the
small weights, and runs a single-pass streaming Bass/Tile kernel per core.

v2: the streamed tensors (x in, gated x out) are staged in bfloat16, halving
HBM traffic on both directions (the kernel is DMA-bound; tolerance is
rel_err < 2e-2 and bf16 staging costs ~1e-3). SE weights/activations run in
bf16 on the PE (FWL fast weight load); pooled sums, EMA scan and the carry
stay in f32.

Algorithm notes:
  - pooled mean and the (1-r) EMA input scale are folded into the first SE
    matmul weights: scan computes q[t] = r*q[t-1] + chunk_sum[t], and
    W1' = fq(w1) * ((1-r)/16) per input channel, so h = s*q never needs to be
    materialized.
  - EMA runs as one hardware TensorTensorScan per (batch, time-block) over the
    flattened (channel-block, pooled-t) axis; r is masked to 0 at each
    channel-block's first pooled step so segments don't leak, and the carry
    from the previous time block is injected into the first chunk-sum.
  - Weight fake-quant (int4 symmetric, round-half-even) is exact host-side
    preprocessing of tiny tensors; all x-dependent compute runs on device.
"""

import contextlib

import numpy as np
import ml_dtypes

import concourse.bacc as bacc
import concourse.mybir as mybir
import concourse.tile as tile
from concourse import bass_utils

F32 = mybir.dt.float32
BF16 = mybir.dt.bfloat16
NP_BF16 = ml_dtypes.bfloat16

B = 16
C = 2048
L = 4096
CHUNK = 16
HID = 256
QMAX = 7
EPS = 1e-5
N_CORES = 8
BPC = B // N_CORES          # batches per core = 2
P = 128
NCB = C // P                # channel blocks = 16
NOC = HID // P              # hidden (SE bottleneck) blocks = 2
TBLK = 1024                 # time elements per streamed block
NTB = L // TBLK             # time blocks per batch
TP = TBLK // CHUNK          # pooled steps per block

_CACHE = {}


def _emit_body(nc, xd, outd, w1, w2, w1b, w2b, onesq, rmasks, rlast,
               xpool, spool, apool, carrypool, ps1, ps2pools, tblk=TBLK,
               ablate=()):
    """One full pass over this core's two batch elements.

    Emission is software-pipelined: each (b, k) iteration emits this block's
    load/pool/scan/SE-gate, but the gate-multiply + store of the PREVIOUS
    block. Engine instruction streams execute in order, so emitting mul(k)
    right after gate(k) would stall the whole DVE stream on the PE/ACT SE
    chain; delaying it one block keeps DVE busy with pooling while the tiny
    SE matmuls for the previous block finish on PE/ACT.
    """
    ntb = L // tblk
    tp = tblk // CHUNK
    HB = NCB // 2  # half of the channel blocks

    pending = None  # (xt, (gate2a, gate2b), b, t0, tb) awaiting mul+store

    def flush_pending(nparts=2):
        nonlocal pending
        if pending is None:
            return
        xt, g2ab, b, t0, tb = pending
        tpk = tb // CHUNK
        # mul+store in channel-block slices: the first slice's gate arrives
        # earlier than the last's, so DVE never waits for the full SE
        # chain, and the store of each slice starts as soon as its mul is
        # done. nparts=8 on the final block shrinks the pipeline drain.
        w = NCB // nparts
        for s in range(nparts):
            cb0 = s * w
            g2 = g2ab[(s * w) // HB]
            g0 = cb0 % HB
            if "mul" not in ablate:
                # view chunk 16 as (8, 2); gate2 holds each gate duplicated
                # x2 so the broadcast operand's innermost dim is dense
                # (2-elem, stride 1) -> DVE 2x_1p mode (all-bf16, packed).
                x5 = xt[:, cb0:cb0 + w, 0:tb].rearrange(
                    "p cb (tp e two) -> p cb tp e two", two=2, e=8)
                g2b = g2[:, g0:g0 + w, 0:tpk].unsqueeze(3).broadcast_to(
                    [P, w, tpk, 8, 2])
                nc.vector.tensor_mul(x5, x5, g2b)
            # stores alternate between the gpsimd and scalar queues so
            # neither ring's serialized DMA stream becomes the bottleneck
            # (the ACT ring only carries them after the mul dep is resolved,
            # well before the next sigmoid is due).
            eng = nc.gpsimd if s % 2 == 0 else nc.scalar
            eng.dma_start(
                outd.ap()[b][cb0:cb0 + w, :, t0:t0 + tb].transpose([1, 0, 2]),
                xt[:, cb0:cb0 + w, 0:tb],
            )
        pending = None

    # 512-wide blocks at the batch edges: the first two shrink the
    # pipeline-fill ramp (pooling starts after a quarter of the bytes), the
    # last two shrink the drain (the final SE-chain latency and mul+store
    # apply to half as much data).
    sched0 = [(0, tblk // 2), (tblk // 2, tblk // 2)] + \
        [(k * tblk, tblk) for k in range(1, ntb)]
    sched1 = [(k * tblk, tblk) for k in range(ntb - 1)] + \
        [((ntb - 1) * tblk, tblk // 2), ((ntb - 1) * tblk + tblk // 2, tblk // 2)]

    for b in range(BPC):
        sched = sched0 if b == 0 else sched1
        qc = carrypool.tile([P, NCB], BF16, tag=f"qc{b}")
        for k, (t0, tb) in enumerate(sched):
            tp_k = tb // CHUNK
            xt = xpool.tile([P, NCB, tblk], BF16, tag="xt")
            # load in channel-block slices so the pooling tree can start
            # after a fraction of the block's bytes have landed; quarters on
            # the very first block to cut the pipeline-fill ramp
            nload = 4 if (b == 0 and k == 0) else 2
            lw = NCB // nload
            for h in range(nload):
                cb0 = h * lw
                nc.sync.dma_start(
                    xt[:, cb0:cb0 + lw, 0:tb],
                    xd.ap()[b][cb0:cb0 + lw, :, t0:t0 + tb].transpose([1, 0, 2]),
                )
            x4 = xt[:, :, 0:tb].rearrange("p cb (tp ch) -> p cb tp ch",
                                          ch=CHUNK)

            # chunk-of-16 sum as a log2 halving tree of dense bf16 TT adds
            # (TensorReduce has no 2x uop mode; dense bf16 adds run 2x).
            # The whole pooled/EMA path stays bf16 so the matmul can read q
            # directly. Tiles are tagged per block width so every scan/merge
            # AP stays contiguous.
            sums = spool.tile([P, NCB, tp_k], BF16, tag=f"sums{tp_k}")
            if "reduce" not in ablate:
                a8 = apool.tile([P, NCB, tp_k, 8], BF16, tag=f"a8{tp_k}")
                if b == 0 and k == 0:
                    # split so pooling starts after the first quarter-load
                    for h in range(4):
                        cb0 = h * (NCB // 4)
                        cw = NCB // 4
                        nc.vector.tensor_add(a8[:, cb0:cb0 + cw],
                                             x4[:, cb0:cb0 + cw, :, 0:8],
                                             x4[:, cb0:cb0 + cw, :, 8:16])
                else:
                    nc.vector.tensor_add(a8[:], x4[:, :, :, 0:8],
                                         x4[:, :, :, 8:16])
                nc.vector.tensor_add(a8[:, :, :, 0:4], a8[:, :, :, 0:4],
                                     a8[:, :, :, 4:8])
                nc.vector.tensor_add(a8[:, :, :, 0:2], a8[:, :, :, 0:2],
                                     a8[:, :, :, 2:4])
                nc.vector.tensor_add(sums[:].unsqueeze(3),
                                     a8[:, :, :, 0:1], a8[:, :, :, 1:2])
            else:
                nc.gpsimd.memset(sums[:], 0.01)

            if k > 0:
                tmp = spool.tile([P, NCB], F32, tag="tmp")
                nc.vector.tensor_mul(tmp[:], qc[:], rlast[:])
                nc.vector.tensor_add(sums[:, :, 0], sums[:, :, 0], tmp[:])

            q = spool.tile([P, NCB, tp_k], BF16, tag=f"q{tp_k}")
            with nc.allow_low_precision(reason="bf16 EMA scan; tolerance 2e-2"):
                nc.vector.tensor_tensor_scan(
                    q[:].rearrange("p cb tp -> p (cb tp)"),
                    (rmasks[0] if tp_k == tp else rmasks[1])[:]
                    .rearrange("p cb tp -> p (cb tp)"),
                    sums[:].rearrange("p cb tp -> p (cb tp)"),
                    initial=0.0,
                    op0=mybir.AluOpType.mult,
                    op1=mybir.AluOpType.add,
                )
            if k < len(sched) - 1:
                nc.vector.tensor_copy(qc[:], q[:, :, tp_k - 1])

            flush_pending()

            # b1/b2 are folded into the matmul accumulation via a rank-1
            # bias matmul (onesq row-0-ones rhs x bias-in-row-0 weights), so
            # relu/sigmoid need no per-oc bias AP and fuse into ONE
            # activation instruction per block (per-inst ACT init dominates
            # otherwise).
            h1 = spool.tile([P, NOC, tp_k], BF16, tag=f"h1{tp_k}")
            acc = ps1.tile([P, NOC, tp_k], F32, tag=f"acc1{tp_k}")
            for oc in range(NOC):
                nc.tensor.matmul(
                    acc[:, oc], w1b[:, oc * P:(oc + 1) * P], onesq[:, 0:tp_k],
                    start=True, stop=False,
                )
                for cb in range(NCB):
                    nc.tensor.matmul(
                        acc[:, oc],
                        w1[:, cb, oc * P:(oc + 1) * P],
                        q[:, cb, :],
                        start=False,
                        stop=(cb == NCB - 1),
                    )
            nc.scalar.activation(
                h1[:].rearrange("p oc tp -> p (oc tp)"),
                acc[:].rearrange("p oc tp -> p (oc tp)"),
                mybir.ActivationFunctionType.Relu,
            )

            # gate duplicated x2 along an innermost dense dim (see flush).
            # The sigmoid reads each PSUM value twice via a stride-0 AP.
            # Emitted in channel-block halves so the first half's gates are
            # ready before DVE drains to the previous block's mul.
            g2ab = []
            for h in range(2):
                gate2 = spool.tile([P, HB, tp_k, 2], BF16, tag=f"gate2{h}_{tp_k}")
                ps2pool = ps2pools[0] if tp_k == tp else ps2pools[1]
                acc2 = ps2pool.tile([P, HB, tp_k], F32, tag=f"acc2{h}_{tp_k}")
                for obl in range(HB):
                    ob = h * HB + obl
                    nc.tensor.matmul(
                        acc2[:, obl], w2b[:, ob * P:(ob + 1) * P],
                        onesq[:, 0:tp_k],
                        start=True, stop=False,
                    )
                    for kc in range(NOC):
                        nc.tensor.matmul(
                            acc2[:, obl],
                            w2[:, kc, ob * P:(ob + 1) * P],
                            h1[:, kc, :],
                            start=False,
                            stop=(kc == NOC - 1),
                        )
                nc.scalar.activation(
                    gate2[:],
                    acc2[:].unsqueeze(3).broadcast_to([P, HB, tp_k, 2]),
                    mybir.ActivationFunctionType.Sigmoid,
                )
                g2ab.append(gate2)

            pending = (xt, tuple(g2ab), b, t0, tb)
    flush_pending(nparts=4)


def _build_module(repeat=1, tblk=TBLK, xbufs=4, sbufs=2, ps1b=1, ps2b=2, ablate=()):
    """Build the per-core module. repeat>1 wraps the body in a hardware loop
    that re-runs it (idempotently) for slope-based device timing."""
    tp = tblk // CHUNK
    nc = bacc.Bacc("TRN2", target_bir_lowering=False, debug=False,
                   num_devices=N_CORES)

    xd = nc.dram_tensor("x", [BPC, NCB, P, L], BF16, kind="ExternalInput")
    w1d = nc.dram_tensor("w1t", [P, NCB, HID], BF16, kind="ExternalInput")
    w2d = nc.dram_tensor("w2t", [P, NOC, C], BF16, kind="ExternalInput")
    w1bd = nc.dram_tensor("w1bm", [P, HID], BF16, kind="ExternalInput")
    w2bd = nc.dram_tensor("w2bm", [P, C], BF16, kind="ExternalInput")
    onesd = nc.dram_tensor("ones_t", [P, tp], BF16, kind="ExternalInput")
    rmd = nc.dram_tensor("rmask", [P, NCB, tp], BF16, kind="ExternalInput")
    rmd2 = nc.dram_tensor("rmask2", [P, NCB, tp // 2], BF16, kind="ExternalInput")
    rld = nc.dram_tensor("rlast", [P, NCB], F32, kind="ExternalInput")
    outd = nc.dram_tensor("out", [BPC, NCB, P, L], BF16, kind="ExternalOutput")

    with tile.TileContext(nc) as tc:
        with (
            tc.tile_pool(name="const", bufs=1) as cpool,
            tc.tile_pool(name="xp", bufs=xbufs) as xpool,
            tc.tile_pool(name="small", bufs=sbufs) as spool,
            tc.tile_pool(name="a8p", bufs=1) as apool,
            tc.tile_pool(name="carry", bufs=1) as carrypool,
            tc.tile_pool(name="ps1", bufs=ps1b, space="PSUM") as ps1,
            tc.tile_pool(name="ps2", bufs=ps2b, space="PSUM") as ps2,
            tc.tile_pool(name="ps2s", bufs=1, space="PSUM") as ps2s,
        ):
            w1 = cpool.tile([P, NCB, HID], BF16)
            w2 = cpool.tile([P, NOC, C], BF16)
            w1b = cpool.tile([P, HID], BF16)
            w2b = cpool.tile([P, C], BF16)
            onesq = cpool.tile([P, tp], BF16)
            rmask = cpool.tile([P, NCB, tp], BF16)
            rmask2 = cpool.tile([P, NCB, tp // 2], BF16)
            rlast = cpool.tile([P, NCB], F32)
            # const loads go on the scalar/ACT queue (idle at startup) so
            # they don't head-block the first x load on the sync queue;
            # scan deps (rmask/rlast) first.
            nc.scalar.dma_start(rmask[:], rmd.ap())
            nc.scalar.dma_start(rmask2[:], rmd2.ap())
            nc.scalar.dma_start(rlast[:], rld.ap())
            nc.scalar.dma_start(onesq[:], onesd.ap())
            nc.scalar.dma_start(w1[:], w1d.ap())
            nc.scalar.dma_start(w1b[:], w1bd.ap())
            # w2/w2b on the gpsimd queue (stores are idle at startup) so the
            # first block's second matmul isn't gated on the ACT-queue tail
            nc.gpsimd.dma_start(w2[:], w2d.ap())
            nc.gpsimd.dma_start(w2b[:], w2bd.ap())

            rep = tc.For_i(0, repeat, 1) if repeat > 1 else contextlib.nullcontext()
            with rep:
                _emit_body(nc, xd, outd, w1, w2, w1b, w2b, onesq,
                           (rmask, rmask2), rlast,
                           xpool, spool, apool, carrypool, ps1,
                           (ps2, ps2s), tblk=tblk, ablate=ablate)

    nc.compile()
    return nc


def _fake_quant(w):
    w = np.asarray(w, np.float32)
    scale = (np.max(np.abs(w), axis=1, keepdims=True).astype(np.float32)
             / np.float32(QMAX) + np.float32(EPS)).astype(np.float32)
    wq = np.clip(np.round(w / scale), -QMAX, QMAX).astype(np.float32) * scale
    return wq.astype(np.float32)


def _host_prep(w1, b1, w2, b2, ema_r, tp=None):
    if tp is None:
        tp = TP
    r = np.asarray(ema_r, np.float32)
    s = ((np.float32(1.0) - r) / np.float32(CHUNK)).astype(np.float32)

    w1s = (_fake_quant(w1) * s[None, :]).astype(np.float32)        # [HID, C]
    w1t = np.ascontiguousarray(
        w1s.T.reshape(NCB, P, HID).transpose(1, 0, 2)).astype(NP_BF16)
    w2q = _fake_quant(w2)                                          # [C, HID]
    w2t = np.ascontiguousarray(
        w2q.T.reshape(NOC, P, C).transpose(1, 0, 2)).astype(NP_BF16)
    # bias fold: rank-1 matmul with onesq (row 0 all ones) x bias-in-row-0
    w1bm = np.zeros((P, HID), np.float32)
    w1bm[0, :] = np.asarray(b1, np.float32)
    w2bm = np.zeros((P, C), np.float32)
    w2bm[0, :] = np.asarray(b2, np.float32)
    ones_t = np.zeros((P, tp), np.float32)
    ones_t[0, :] = 1.0

    rpb = r.reshape(NCB, P).T                                      # [P, NCB]
    rmask = np.repeat(rpb[:, :, None], tp, axis=2).astype(np.float32)
    rmask[:, :, 0] = 0.0
    rmask2 = np.ascontiguousarray(rmask[:, :, 0:tp // 2])
    rlast = np.ascontiguousarray(rpb)
    return (w1t, w2t, w1bm.astype(NP_BF16), w2bm.astype(NP_BF16),
            ones_t.astype(NP_BF16),
            np.ascontiguousarray(rmask).astype(NP_BF16),
            rmask2.astype(NP_BF16), rlast)


def _make_in_maps(x, w1, b1, w2, b2, ema_r, tp=None):
    w1t, w2t, w1bm, w2bm, ones_t, rmask, rmask2, rlast = _host_prep(
        w1, b1, w2, b2, ema_r, tp=tp)
    xh = np.asarray(x, np.float32).reshape(B, NCB, P, L).astype(NP_BF16)
    return [{
        "x": xh[c * BPC:(c + 1) * BPC],
        "w1t": w1t, "w2t": w2t, "w1bm": w1bm, "w2bm": w2bm,
        "ones_t": ones_t, "rmask": rmask, "rmask2": rmask2, "rlast": rlast,
    } for c in range(N_CORES)]


def kernel(x, w1, b1, w2, b2, ema_r):
    if "nc" not in _CACHE:
        _CACHE["nc"] = _build_module()
    nc = _CACHE["nc"]

    in_maps = _make_in_maps(x, w1, b1, w2, b2, ema_r)
    res = bass_utils.run_bass_kernel_spmd(nc, in_maps,
                                          core_ids=list(range(N_CORES)))
    out = np.empty((B, NCB, P, L), np.float32)
    for c in range(N_CORES):
        out[c * BPC:(c + 1) * BPC] = res.results[c]["out"]
    return out.reshape(B, C, L)
